# revision 9
# baseline (speedup 1.0000x reference)
"""MoE (top-2 of 8 experts, swiglu MLP) Trainium2 kernel.

Strategy (expert parallelism, per the sharding hint):
  - Host computes the gate in float64 (scores = x @ gate_w.T, top-2,
    softmax over the selected pair) and dispatches each token to its two
    experts: this is the "all-to-all by top-k expert index" shard step.
  - Core e receives expert e's weights plus the gathered tokens routed to
    it (transposed, [D, C] with C a common padded capacity, fp16) and
    computes  y = (silu(x @ w1) * (x @ w3)) @ w2 * coef[token]
    on device: fp16 matmul operands, fp32 PSUM accumulation.
  - Host scatter-adds each expert's [C, D] (fp16) result back into the
    fp32 output.

Device kernel is a single pass: x ([128, 8, C] f16) and the swiglu
activation g ([128, 22, C] f16) stay SBUF-resident, w13 is streamed once
(per h-tile), w2 is resident.  Startup latency is hidden by splitting the
x load into (ds x col-sub) pieces across three DMA queues and by warmup
matmuls that lift the PE HAM clock gate to 8/8 before real data lands.

Shapes: B=4, S=2048, D=1024, H=2816, E=8, K=2.
"""

import numpy as np

B, S, D, HID, E, TOPK = 4, 2048, 1024, 2816, 8, 2
P = 128
DSUB = D // P
HT = HID // P
NCORES = 8

_nc_cache: dict[int, object] = {}


def _subs(C):
    # Column sub-tiles, all >= 256 wide where possible (N=128 streams at
    # ~0.66 ns/col vs ~0.42 for N>=256): a 128 remainder is folded by
    # replacing the last 512 with 384+256.  The first two subs are 256
    # wide so the first matmul group waits on a small x DMA footprint.
    out, rem = [], C
    while rem:
        s = 384 if (rem % 512 == 128 and rem > 128) else min(512, rem)
        out.append(s)
        rem -= s
    # smallest first: the opening matmul group then waits on the
    # smallest x DMA footprint
    return sorted(out)


def _build(C):
    import concourse.tile as tile
    from concourse import bacc, mybir

    F32, F16 = mybir.dt.float32, mybir.dt.float16
    SILU = mybir.ActivationFunctionType.Silu
    MULT = mybir.AluOpType.mult

    nc = bacc.Bacc("TRN2", target_bir_lowering=False, debug=False,
                   num_devices=NCORES)
    xT = nc.dram_tensor("xT", [D, C], F16, kind="ExternalInput")
    w13 = nc.dram_tensor("w13", [D, HT, 2 * P], F16, kind="ExternalInput")
    w2 = nc.dram_tensor("w2", [HID, D], F16, kind="ExternalInput")
    coef = nc.dram_tensor("coef", [P, C // P], F32, kind="ExternalInput")
    y = nc.dram_tensor("y", [C, D], F16, kind="ExternalOutput")

    xT_r = xT.ap().rearrange("(do dp) c -> dp do c", dp=P)
    w13_r = w13.ap().rearrange("(do dp) ht z -> dp do ht z", dp=P)
    w2_r = w2.ap().rearrange("(ho hp) d -> hp ho d", hp=P)

    subs = _subs(C)
    sub_lo = [sum(subs[:i]) for i in range(len(subs))]
    xq = [nc.gpsimd, nc.scalar]   # queues allowed to initiate x DMAs

    with tile.TileContext(nc) as tc:
        with tc.tile_pool(name="wts", bufs=1) as wts, \
             tc.tile_pool(name="wpool", bufs=3) as wpool, \
             tc.tile_pool(name="tpool", bufs=2) as tpool, \
             tc.tile_pool(name="ypool", bufs=2) as ypool, \
             tc.tile_pool(name="psum", bufs=2, space="PSUM") as psum, \
             tc.tile_pool(name="psum2", bufs=2, space="PSUM") as psum2:
            x_sb = wts.tile([P, DSUB, C], F16, tag="x")
            g = wts.tile([P, HT, C], F16, tag="g")
            w2_sb = wts.tile([P, HT, D], F16, tag="w2")
            coef_sb = wts.tile([P, C // P], F32, tag="coef")
            dum = wts.tile([P, P], F16, tag="dum")

            # -- warmup: lift the HAM clock gate while the first DMAs run
            nc.vector.memset(dum[:], 0.0)
            for i in range(12):
                pw = psum2.tile([P, 512], F32, tag="py")
                nc.tensor.matmul(pw[:, :P], dum[:], dum[:],
                                 start=True, stop=True)

            nc.gpsimd.dma_start(coef_sb[:], coef.ap())
            # x pieces per (sub, ds), sub-major: each col-sub's matmul
            # group unblocks as soon as its own 8 pieces land.
            qi = 0
            for lo, w in zip(sub_lo, subs):
                for ds_ in range(DSUB):
                    xq[qi % 2].dma_start(x_sb[:, ds_, lo:lo + w],
                                         xT_r[:, ds_, lo:lo + w])
                    qi += 1

            for ht in range(HT):
                wc = wpool.tile([P, DSUB, 2 * P], F16, tag="w13")
                # split per projection so the first matmul group only
                # waits on the w1 half
                nc.sync.dma_start(wc[:, :, :P], w13_r[:, :, ht, :P])
                nc.sync.dma_start(wc[:, :, P:], w13_r[:, :, ht, P:])
                nc.scalar.dma_start(w2_sb[:, ht, :], w2_r[:, ht, :])
                for lo, w in zip(sub_lo, subs):
                    ph1 = psum.tile([P, 512], F32, tag="ph1")
                    ph3 = psum.tile([P, 512], F32, tag="ph3")
                    for ds_ in range(DSUB):
                        nc.tensor.matmul(
                            ph1[:, :w], wc[:, ds_, :P],
                            x_sb[:, ds_, lo:lo + w],
                            start=(ds_ == 0), stop=(ds_ == DSUB - 1))
                    for ds_ in range(DSUB):
                        nc.tensor.matmul(
                            ph3[:, :w], wc[:, ds_, P:],
                            x_sb[:, ds_, lo:lo + w],
                            start=(ds_ == 0), stop=(ds_ == DSUB - 1))
                    tmp = tpool.tile([P, 512], F16, tag="tmp")
                    nc.scalar.activation(tmp[:, :w], ph1[:, :w], SILU)
                    nc.vector.tensor_tensor(g[:, ht, lo:lo + w],
                                            tmp[:, :w], ph3[:, :w], MULT)

            ncs = C // P
            for cs in range(ncs):
                for dt_ in range(D // 512):
                    py_ = psum2.tile([P, 512], F32, tag="py")
                    for ht in range(HT):
                        nc.tensor.matmul(
                            py_[:], g[:, ht, cs * P:(cs + 1) * P],
                            w2_sb[:, ht, dt_ * 512:(dt_ + 1) * 512],
                            start=(ht == 0), stop=(ht == HT - 1))
                    last = cs == ncs - 1 and dt_ == D // 512 - 1
                    # split the final drain so the tail DVE op + y DMA
                    # pipeline instead of serializing after the last MM
                    for p0, pw_ in ([(0, 256), (256, 256)] if last
                                    else [(0, 512)]):
                        ysb = ypool.tile([P, 512], F16, tag="y")
                        nc.vector.tensor_scalar_mul(
                            ysb[:, :pw_], py_[:, p0:p0 + pw_],
                            coef_sb[:, cs, None])
                        nc.scalar.dma_start(
                            y.ap()[cs * P:(cs + 1) * P,
                                   dt_ * 512 + p0:dt_ * 512 + p0 + pw_],
                            ysb[:, :pw_])
    nc.compile()
    return nc


def _get_nc(C):
    if C not in _nc_cache:
        _nc_cache[C] = _build(C)
    return _nc_cache[C]


def _route(xt, gate_w):
    T = xt.shape[0]
    scores = xt.astype(np.float64) @ gate_w.astype(np.float64).T
    ar = np.arange(T)
    i1 = np.argmax(scores, 1)
    s1 = scores[ar, i1]
    scores[ar, i1] = -np.inf
    i2 = np.argmax(scores, 1)
    s2 = scores[ar, i2]
    e2 = np.exp(s2 - s1)
    denom = 1.0 + e2
    return i1, i2, 1.0 / denom, e2 / denom


def _ensure_axon_hooks():
    """bass_utils imports antenv.axon_hooks when tracing is requested
    (e.g. BASS_TRACE=1); some images lack that module. Register a shim
    backed by the boot ctypes NTFF hook so tracing works instead of
    crashing."""
    try:
        import antenv.axon_hooks  # noqa: F401
        return
    except ImportError:
        pass
    import sys
    import types
    hook = None
    try:
        from trn_agent_boot.trn_boot import _ntff_profile_via_ctypes
        hook = _ntff_profile_via_ctypes("/opt/axon/libaxon_pjrt.so")
    except Exception:
        hook = None
    try:
        import antenv
    except ImportError:
        return
    mod = types.ModuleType("antenv.axon_hooks")
    mod.get_axon_ntff_profile_hook = lambda: hook
    mod.set_axon_ntff_profile_hook = lambda h: None
    sys.modules["antenv.axon_hooks"] = mod
    antenv.axon_hooks = mod


def kernel(x, gate_w, w1, w3, w2):
    _ensure_axon_hooks()
    from concourse.bass_utils import run_bass_kernel_spmd

    x = np.asarray(x, dtype=np.float32)
    gate_w = np.asarray(gate_w, dtype=np.float32)
    w1 = np.asarray(w1, dtype=np.float32)
    w3 = np.asarray(w3, dtype=np.float32)
    w2 = np.asarray(w2, dtype=np.float32)

    b, s, d = x.shape
    T = b * s
    xt = x.reshape(T, d)
    i1, i2, wa, wb = _route(xt, gate_w)

    idxs, coefs = [], []
    for e in range(E):
        m1 = i1 == e
        m2 = i2 == e
        cf = np.where(m1, wa, 0.0) + np.where(m2, wb, 0.0)
        idx = np.nonzero(m1 | m2)[0]
        idxs.append(idx)
        coefs.append(cf[idx].astype(np.float32))

    maxc = max(len(i) for i in idxs)
    C = max(256, -(-maxc // 128) * 128)
    nc = _get_nc(C)

    xtT = np.ascontiguousarray(xt.T.astype(np.float16))
    in_maps = []
    for e in range(E):
        n = len(idxs[e])
        xTe = np.zeros((D, C), np.float16)
        xTe[:, :n] = xtT[:, idxs[e]]
        cfull = np.zeros(C, np.float32)
        cfull[:n] = coefs[e]
        coef2d = np.ascontiguousarray(cfull.reshape(C // P, P).T)
        w13 = np.empty((D, HT, 2 * P), np.float16)
        w13[:, :, :P] = w1[e].reshape(D, HT, P)
        w13[:, :, P:] = w3[e].reshape(D, HT, P)
        in_maps.append({
            "xT": xTe,
            "w13": w13,
            "w2": np.ascontiguousarray(w2[e].astype(np.float16)),
            "coef": coef2d,
        })

    res = run_bass_kernel_spmd(nc, in_maps, core_ids=list(range(NCORES)))

    out = np.zeros((T, D), np.float32)
    for e in range(E):
        n = len(idxs[e])
        out[idxs[e]] += res.results[e]["y"][:n].astype(np.float32)
    return out.reshape(b, s, d)


# revision 12
# speedup vs baseline: 1.0126x; 1.0126x over previous
"""MoE (top-2 of 8 experts, swiglu MLP) Trainium2 kernel.

Strategy (expert parallelism, per the sharding hint):
  - Host computes the gate in float64 (scores = x @ gate_w.T, top-2,
    softmax over the selected pair) and dispatches each token to its two
    experts: this is the "all-to-all by top-k expert index" shard step.
  - Core e receives expert e's weights plus the gathered tokens routed to
    it (transposed, [D, C] with C a common padded capacity, fp16) and
    computes  y = (silu(x @ w1) * (x @ w3)) @ w2 * coef[token]
    on device: fp16 matmul operands, fp32 PSUM accumulation.
  - Host scatter-adds each expert's [C, D] (fp16) result back into the
    fp32 output.

Device kernel is a single pass: x ([128, 8, C] f16) and the swiglu
activation g ([128, 22, C] f16) stay SBUF-resident, w13 is streamed once
(per h-tile), w2 is resident.  Startup latency is hidden by splitting the
x load into (ds x col-sub) pieces across three DMA queues and by warmup
matmuls that lift the PE HAM clock gate to 8/8 before real data lands.

Shapes: B=4, S=2048, D=1024, H=2816, E=8, K=2.
"""

import numpy as np

B, S, D, HID, E, TOPK = 4, 2048, 1024, 2816, 8, 2
P = 128
DSUB = D // P
HT = HID // P
NCORES = 8

_nc_cache: dict[int, object] = {}


def _subs(C):
    # Column sub-tiles, all >= 256 wide where possible (N=128 streams at
    # ~0.66 ns/col vs ~0.42 for N>=256): a 128 remainder is folded by
    # replacing the last 512 with 384+256.  The first two subs are 256
    # wide so the first matmul group waits on a small x DMA footprint.
    out, rem = [], C
    while rem:
        s = 384 if (rem % 512 == 128 and rem > 128) else min(512, rem)
        out.append(s)
        rem -= s
    # smallest first: the opening matmul group then waits on the
    # smallest x DMA footprint
    return sorted(out)


def _build(C):
    import concourse.tile as tile
    from concourse import bacc, mybir

    F32, F16 = mybir.dt.float32, mybir.dt.float16
    SILU = mybir.ActivationFunctionType.Silu
    MULT = mybir.AluOpType.mult

    nc = bacc.Bacc("TRN2", target_bir_lowering=False, debug=False,
                   num_devices=NCORES)
    xT = nc.dram_tensor("xT", [D, C], F16, kind="ExternalInput")
    # w13 packed host-side as [ht, half, dp, do*z]: each (ht, half) slice
    # is a 2 KiB-contiguous line per partition (256B lines of the naive
    # layout measured only ~37 GB/s on the sync queue)
    w13 = nc.dram_tensor("w13", [HT, 2, P, DSUB * P], F16,
                         kind="ExternalInput")
    w2 = nc.dram_tensor("w2", [HID, D], F16, kind="ExternalInput")
    coef = nc.dram_tensor("coef", [P, C // P], F32, kind="ExternalInput")
    y = nc.dram_tensor("y", [C, D], F16, kind="ExternalOutput")

    xT_r = xT.ap().rearrange("(do dp) c -> dp do c", dp=P)
    w13_r = w13.ap()
    w2_r = w2.ap().rearrange("(ho hp) d -> hp ho d", hp=P)

    subs = _subs(C)
    sub_lo = [sum(subs[:i]) for i in range(len(subs))]
    xq = [nc.gpsimd, nc.scalar, nc.sync]  # x DMA queues, round-robin

    with tile.TileContext(nc) as tc:
        with tc.tile_pool(name="wts", bufs=1) as wts, \
             tc.tile_pool(name="wpool", bufs=3) as wpool, \
             tc.tile_pool(name="tpool", bufs=2) as tpool, \
             tc.tile_pool(name="ypool", bufs=2) as ypool, \
             tc.tile_pool(name="psum", bufs=2, space="PSUM") as psum, \
             tc.tile_pool(name="psum2", bufs=2, space="PSUM") as psum2:
            x_sb = wts.tile([P, DSUB, C], F16, tag="x")
            g = wts.tile([P, HT, C], F16, tag="g")
            w2_sb = wts.tile([P, HT, D], F16, tag="w2")
            coef_sb = wts.tile([P, C // P], F32, tag="coef")
            dum = wts.tile([P, P], F16, tag="dum")

            # -- warmup: lift the HAM clock gate while the first DMAs run
            nc.vector.memset(dum[:], 0.0)
            for i in range(12):
                pw = psum2.tile([P, 512], F32, tag="py")
                nc.tensor.matmul(pw[:, :P], dum[:], dum[:],
                                 start=True, stop=True)

            wcs = {}

            def get_wc(ht):
                if ht not in wcs:
                    wc = wpool.tile([P, 2, DSUB * P], F16, tag="w13")
                    nc.sync.dma_start(wc[:, 0, :], w13_r[ht, 0])
                    nc.sync.dma_start(wc[:, 1, :], w13_r[ht, 1])
                    nc.scalar.dma_start(w2_sb[:, ht, :], w2_r[:, ht, :])
                    wcs[ht] = wc
                return wcs[ht]

            # w13 for the first rows dispatches ahead of the x pieces on
            # the sync queue (queue FIFO: later waits would block x)
            for ht in range(min(3, HT)):
                get_wc(ht)

            nc.gpsimd.dma_start(coef_sb[:], coef.ap())
            # x pieces per (sub, ds-pair), sub-major, across all three
            # DMA queues: a col-sub's matmuls unblock when its 4 pieces
            # land.
            qi = 0
            for lo, w in zip(sub_lo, subs):
                for dp_ in range(DSUB // 2):
                    xq[qi % 3].dma_start(
                        x_sb[:, 2 * dp_:2 * dp_ + 2, lo:lo + w],
                        xT_r[:, 2 * dp_:2 * dp_ + 2, lo:lo + w])
                    qi += 1

            # (ht, sub) pair order: defer the tail subs of ht0/ht1 so the
            # opening matmuls only need the first col-subs of x
            nsub = len(subs)
            if HT >= 2 and nsub > 3:
                pairs = [(0, s) for s in range(3)] \
                    + [(1, s) for s in range(3)] \
                    + [(0, s) for s in range(3, nsub)] \
                    + [(1, s) for s in range(3, nsub)] \
                    + [(ht, s) for ht in range(2, HT) for s in range(nsub)]
            else:
                pairs = [(ht, s) for ht in range(HT) for s in range(nsub)]

            for ht, si in pairs:
                wc = get_wc(ht)
                lo, w = sub_lo[si], subs[si]
                ph1 = psum.tile([P, 512], F32, tag="ph1")
                ph3 = psum.tile([P, 512], F32, tag="ph3")
                for ds_ in range(DSUB):
                    nc.tensor.matmul(
                        ph1[:, :w], wc[:, 0, ds_ * P:(ds_ + 1) * P],
                        x_sb[:, ds_, lo:lo + w],
                        start=(ds_ == 0), stop=(ds_ == DSUB - 1))
                for ds_ in range(DSUB):
                    nc.tensor.matmul(
                        ph3[:, :w], wc[:, 1, ds_ * P:(ds_ + 1) * P],
                        x_sb[:, ds_, lo:lo + w],
                        start=(ds_ == 0), stop=(ds_ == DSUB - 1))
                tmp = tpool.tile([P, 512], F16, tag="tmp")
                nc.scalar.activation(tmp[:, :w], ph1[:, :w], SILU)
                nc.vector.tensor_tensor(g[:, ht, lo:lo + w],
                                        tmp[:, :w], ph3[:, :w], MULT)

            ncs = C // P
            for cs in range(ncs):
                for dt_ in range(D // 512):
                    py_ = psum2.tile([P, 512], F32, tag="py")
                    for ht in range(HT):
                        nc.tensor.matmul(
                            py_[:], g[:, ht, cs * P:(cs + 1) * P],
                            w2_sb[:, ht, dt_ * 512:(dt_ + 1) * 512],
                            start=(ht == 0), stop=(ht == HT - 1))
                    last = cs == ncs - 1 and dt_ == D // 512 - 1
                    # split the final drain so the tail DVE op + y DMA
                    # pipeline instead of serializing after the last MM
                    for p0, pw_ in ([(0, 256), (256, 256)] if last
                                    else [(0, 512)]):
                        ysb = ypool.tile([P, 512], F16, tag="y")
                        nc.vector.tensor_scalar_mul(
                            ysb[:, :pw_], py_[:, p0:p0 + pw_],
                            coef_sb[:, cs, None])
                        nc.scalar.dma_start(
                            y.ap()[cs * P:(cs + 1) * P,
                                   dt_ * 512 + p0:dt_ * 512 + p0 + pw_],
                            ysb[:, :pw_])
    nc.compile()
    return nc


def _get_nc(C):
    if C not in _nc_cache:
        _nc_cache[C] = _build(C)
    return _nc_cache[C]


def _route(xt, gate_w):
    T = xt.shape[0]
    scores = xt.astype(np.float64) @ gate_w.astype(np.float64).T
    ar = np.arange(T)
    i1 = np.argmax(scores, 1)
    s1 = scores[ar, i1]
    scores[ar, i1] = -np.inf
    i2 = np.argmax(scores, 1)
    s2 = scores[ar, i2]
    e2 = np.exp(s2 - s1)
    denom = 1.0 + e2
    return i1, i2, 1.0 / denom, e2 / denom


def _ensure_axon_hooks():
    """bass_utils imports antenv.axon_hooks when tracing is requested
    (e.g. BASS_TRACE=1); some images lack that module. Register a shim
    backed by the boot ctypes NTFF hook so tracing works instead of
    crashing."""
    try:
        import antenv.axon_hooks  # noqa: F401
        return
    except ImportError:
        pass
    import sys
    import types
    hook = None
    try:
        from trn_agent_boot.trn_boot import _ntff_profile_via_ctypes
        hook = _ntff_profile_via_ctypes("/opt/axon/libaxon_pjrt.so")
    except Exception:
        hook = None
    try:
        import antenv
    except ImportError:
        return
    mod = types.ModuleType("antenv.axon_hooks")
    mod.get_axon_ntff_profile_hook = lambda: hook
    mod.set_axon_ntff_profile_hook = lambda h: None
    sys.modules["antenv.axon_hooks"] = mod
    antenv.axon_hooks = mod


def kernel(x, gate_w, w1, w3, w2):
    _ensure_axon_hooks()
    from concourse.bass_utils import run_bass_kernel_spmd

    x = np.asarray(x, dtype=np.float32)
    gate_w = np.asarray(gate_w, dtype=np.float32)
    w1 = np.asarray(w1, dtype=np.float32)
    w3 = np.asarray(w3, dtype=np.float32)
    w2 = np.asarray(w2, dtype=np.float32)

    b, s, d = x.shape
    T = b * s
    xt = x.reshape(T, d)
    i1, i2, wa, wb = _route(xt, gate_w)

    idxs, coefs = [], []
    for e in range(E):
        m1 = i1 == e
        m2 = i2 == e
        cf = np.where(m1, wa, 0.0) + np.where(m2, wb, 0.0)
        idx = np.nonzero(m1 | m2)[0]
        idxs.append(idx)
        coefs.append(cf[idx].astype(np.float32))

    maxc = max(len(i) for i in idxs)
    C = max(256, -(-maxc // 128) * 128)
    nc = _get_nc(C)

    xtT = np.ascontiguousarray(xt.T.astype(np.float16))
    in_maps = []
    for e in range(E):
        n = len(idxs[e])
        xTe = np.zeros((D, C), np.float16)
        xTe[:, :n] = xtT[:, idxs[e]]
        cfull = np.zeros(C, np.float32)
        cfull[:n] = coefs[e]
        coef2d = np.ascontiguousarray(cfull.reshape(C // P, P).T)
        # [ht, half, dp, do*z]: per-(ht, half) slices are contiguous
        # 2 KiB lines per partition
        w13 = np.empty((HT, 2, P, DSUB * P), np.float16)
        w13[:, 0] = w1[e].reshape(DSUB, P, HT, P).transpose(2, 1, 0, 3) \
                         .reshape(HT, P, DSUB * P)
        w13[:, 1] = w3[e].reshape(DSUB, P, HT, P).transpose(2, 1, 0, 3) \
                         .reshape(HT, P, DSUB * P)
        in_maps.append({
            "xT": xTe,
            "w13": w13,
            "w2": np.ascontiguousarray(w2[e].astype(np.float16)),
            "coef": coef2d,
        })

    res = run_bass_kernel_spmd(nc, in_maps, core_ids=list(range(NCORES)))

    out = np.zeros((T, D), np.float32)
    for e in range(E):
        n = len(idxs[e])
        out[idxs[e]] += res.results[e]["y"][:n].astype(np.float32)
    return out.reshape(b, s, d)


# revision 15
# speedup vs baseline: 1.0256x; 1.0129x over previous
"""MoE (top-2 of 8 experts, swiglu MLP) Trainium2 kernel.

Strategy (expert parallelism, per the sharding hint):
  - Host computes the gate in float64 (scores = x @ gate_w.T, top-2,
    softmax over the selected pair) and dispatches each token to its two
    experts: this is the "all-to-all by top-k expert index" shard step.
  - Core e receives expert e's weights plus the gathered tokens routed to
    it (transposed, [D, C] with C a common padded capacity, fp16) and
    computes  y = (silu(x @ w1) * (x @ w3)) @ w2 * coef[token]
    on device: fp16 matmul operands, fp32 PSUM accumulation.
  - Host scatter-adds each expert's [C, D] (fp16) result back into the
    fp32 output.

Device kernel is a single pass: x ([128, 8, C] f16) and the swiglu
activation g ([128, 22, C] f16) stay SBUF-resident, w13 is streamed once
(per h-tile), w2 is resident.  Startup latency is hidden by splitting the
x load into (ds x col-sub) pieces across three DMA queues and by warmup
matmuls that lift the PE HAM clock gate to 8/8 before real data lands.

Shapes: B=4, S=2048, D=1024, H=2816, E=8, K=2.
"""

import numpy as np

B, S, D, HID, E, TOPK = 4, 2048, 1024, 2816, 8, 2
P = 128
DSUB = D // P
HT = HID // P
NCORES = 8

_nc_cache: dict[int, object] = {}


def _subs(C):
    # Column sub-tiles, all >= 256 wide where possible (N=128 streams at
    # ~0.66 ns/col vs ~0.42 for N>=256): a 128 remainder is folded by
    # replacing the last 512 with 384+256.  The first two subs are 256
    # wide so the first matmul group waits on a small x DMA footprint.
    out, rem = [], C
    while rem:
        s = 384 if (rem % 512 == 128 and rem > 128) else min(512, rem)
        out.append(s)
        rem -= s
    # smallest first: the opening matmul group then waits on the
    # smallest x DMA footprint
    return sorted(out)


def _build(C):
    import concourse.tile as tile
    from concourse import bacc, mybir

    F32, F16 = mybir.dt.float32, mybir.dt.float16
    SILU = mybir.ActivationFunctionType.Silu
    MULT = mybir.AluOpType.mult

    nc = bacc.Bacc("TRN2", target_bir_lowering=False, debug=False,
                   num_devices=NCORES)
    xT = nc.dram_tensor("xT", [D, C], F16, kind="ExternalInput")
    # w13 packed host-side as [ht, half, dp, do*z]: each (ht, half) slice
    # is a 2 KiB-contiguous line per partition (256B lines of the naive
    # layout measured only ~37 GB/s on the sync queue)
    w13 = nc.dram_tensor("w13", [HT, 2, P, DSUB * P], F16,
                         kind="ExternalInput")
    w2 = nc.dram_tensor("w2", [HID, D], F16, kind="ExternalInput")
    coef = nc.dram_tensor("coef", [P, C // P], F32, kind="ExternalInput")
    y = nc.dram_tensor("y", [C, D], F16, kind="ExternalOutput")

    xT_r = xT.ap().rearrange("(do dp) c -> dp do c", dp=P)
    w13_r = w13.ap()
    w2_r = w2.ap().rearrange("(ho hp) d -> hp ho d", hp=P)

    subs = _subs(C)
    sub_lo = [sum(subs[:i]) for i in range(len(subs))]

    with tile.TileContext(nc) as tc:
        with tc.tile_pool(name="wts", bufs=1) as wts, \
             tc.tile_pool(name="wpool", bufs=3) as wpool, \
             tc.tile_pool(name="tpool", bufs=2) as tpool, \
             tc.tile_pool(name="ypool", bufs=2) as ypool, \
             tc.tile_pool(name="psum", bufs=2, space="PSUM") as psum, \
             tc.tile_pool(name="psum2", bufs=2, space="PSUM") as psum2:
            x_sb = wts.tile([P, DSUB, C], F16, tag="x")
            g = wts.tile([P, HT, C], F16, tag="g")
            w2_sb = wts.tile([P, HT, D], F16, tag="w2")
            coef_sb = wts.tile([P, C // P], F32, tag="coef")
            dum = wts.tile([P, P], F16, tag="dum")

            # -- warmup: lift the HAM clock gate while the first DMAs run
            nc.vector.memset(dum[:], 0.0)
            for i in range(12):
                pw = psum2.tile([P, 512], F32, tag="py")
                nc.tensor.matmul(pw[:, :P], dum[:], dum[:],
                                 start=True, stop=True)

            wcs = {}

            def get_wc(ht):
                # sync queue carries ONLY the w13 stream: one ht tile
                # every ~3.4us; x shares a queue here would delay it
                if ht not in wcs:
                    wc = wpool.tile([P, 2, DSUB * P], F16, tag="w13")
                    nc.sync.dma_start(wc[:, 0, :], w13_r[ht, 0])
                    nc.sync.dma_start(wc[:, 1, :], w13_r[ht, 1])
                    wcs[ht] = wc
                return wcs[ht]

            for ht in range(min(2, HT)):
                get_wc(ht)

            nc.gpsimd.dma_start(coef_sb[:], coef.ap())
            # x pieces: per sub, ds 0-3 on scalar / ds 4-7 on gpsimd
            for lo, w in zip(sub_lo, subs):
                nc.scalar.dma_start(x_sb[:, :DSUB // 2, lo:lo + w],
                                    xT_r[:, :DSUB // 2, lo:lo + w])
                nc.gpsimd.dma_start(x_sb[:, DSUB // 2:, lo:lo + w],
                                    xT_r[:, DSUB // 2:, lo:lo + w])
            # w2 is not needed until stage 2: fetch on gpsimd after x
            for ht in range(HT):
                nc.gpsimd.dma_start(w2_sb[:, ht, :], w2_r[:, ht, :])

            # (ht, sub) pair order: wavefront over the first three ht
            # rows matched to the x/w13 DMA arrival order, then row-major
            nsub = len(subs)
            if HT > 3 and nsub > 3:
                pairs = [(0, 0), (0, 1), (1, 0), (1, 1), (2, 0), (2, 1),
                         (0, 2), (1, 2), (2, 2)]
                pairs += [(h, s) for s in range(3, nsub) for h in range(3)]
                pairs += [(h, s) for h in range(3, HT) for s in range(nsub)]
            else:
                pairs = [(ht, s) for ht in range(HT) for s in range(nsub)]

            for ht, si in pairs:
                wc = get_wc(ht)
                lo, w = sub_lo[si], subs[si]
                ph1 = psum.tile([P, 512], F32, tag="ph1")
                ph3 = psum.tile([P, 512], F32, tag="ph3")
                for ds_ in range(DSUB):
                    nc.tensor.matmul(
                        ph1[:, :w], wc[:, 0, ds_ * P:(ds_ + 1) * P],
                        x_sb[:, ds_, lo:lo + w],
                        start=(ds_ == 0), stop=(ds_ == DSUB - 1))
                for ds_ in range(DSUB):
                    nc.tensor.matmul(
                        ph3[:, :w], wc[:, 1, ds_ * P:(ds_ + 1) * P],
                        x_sb[:, ds_, lo:lo + w],
                        start=(ds_ == 0), stop=(ds_ == DSUB - 1))
                tmp = tpool.tile([P, 512], F16, tag="tmp")
                nc.scalar.activation(tmp[:, :w], ph1[:, :w], SILU)
                nc.vector.tensor_tensor(g[:, ht, lo:lo + w],
                                        tmp[:, :w], ph3[:, :w], MULT)

            ncs = C // P
            for cs in range(ncs):
                for dt_ in range(D // 512):
                    py_ = psum2.tile([P, 512], F32, tag="py")
                    for ht in range(HT):
                        nc.tensor.matmul(
                            py_[:], g[:, ht, cs * P:(cs + 1) * P],
                            w2_sb[:, ht, dt_ * 512:(dt_ + 1) * 512],
                            start=(ht == 0), stop=(ht == HT - 1))
                    ysb = ypool.tile([P, 512], F16, tag="y")
                    nc.vector.tensor_scalar_mul(ysb[:], py_[:],
                                                coef_sb[:, cs, None])
                    nc.scalar.dma_start(
                        y.ap()[cs * P:(cs + 1) * P,
                               dt_ * 512:(dt_ + 1) * 512], ysb[:])
    nc.compile()
    return nc


def _get_nc(C):
    if C not in _nc_cache:
        _nc_cache[C] = _build(C)
    return _nc_cache[C]


def _route(xt, gate_w):
    T = xt.shape[0]
    scores = xt.astype(np.float64) @ gate_w.astype(np.float64).T
    ar = np.arange(T)
    i1 = np.argmax(scores, 1)
    s1 = scores[ar, i1]
    scores[ar, i1] = -np.inf
    i2 = np.argmax(scores, 1)
    s2 = scores[ar, i2]
    e2 = np.exp(s2 - s1)
    denom = 1.0 + e2
    return i1, i2, 1.0 / denom, e2 / denom


def _ensure_axon_hooks():
    """bass_utils imports antenv.axon_hooks when tracing is requested
    (e.g. BASS_TRACE=1); some images lack that module. Register a shim
    backed by the boot ctypes NTFF hook so tracing works instead of
    crashing."""
    try:
        import antenv.axon_hooks  # noqa: F401
        return
    except ImportError:
        pass
    import sys
    import types
    hook = None
    try:
        from trn_agent_boot.trn_boot import _ntff_profile_via_ctypes
        hook = _ntff_profile_via_ctypes("/opt/axon/libaxon_pjrt.so")
    except Exception:
        hook = None
    try:
        import antenv
    except ImportError:
        return
    mod = types.ModuleType("antenv.axon_hooks")
    mod.get_axon_ntff_profile_hook = lambda: hook
    mod.set_axon_ntff_profile_hook = lambda h: None
    sys.modules["antenv.axon_hooks"] = mod
    antenv.axon_hooks = mod


def kernel(x, gate_w, w1, w3, w2):
    _ensure_axon_hooks()
    from concourse.bass_utils import run_bass_kernel_spmd

    x = np.asarray(x, dtype=np.float32)
    gate_w = np.asarray(gate_w, dtype=np.float32)
    w1 = np.asarray(w1, dtype=np.float32)
    w3 = np.asarray(w3, dtype=np.float32)
    w2 = np.asarray(w2, dtype=np.float32)

    b, s, d = x.shape
    T = b * s
    xt = x.reshape(T, d)
    i1, i2, wa, wb = _route(xt, gate_w)

    idxs, coefs = [], []
    for e in range(E):
        m1 = i1 == e
        m2 = i2 == e
        cf = np.where(m1, wa, 0.0) + np.where(m2, wb, 0.0)
        idx = np.nonzero(m1 | m2)[0]
        idxs.append(idx)
        coefs.append(cf[idx].astype(np.float32))

    maxc = max(len(i) for i in idxs)
    C = max(256, -(-maxc // 128) * 128)
    nc = _get_nc(C)

    xtT = np.ascontiguousarray(xt.T.astype(np.float16))
    in_maps = []
    for e in range(E):
        n = len(idxs[e])
        xTe = np.zeros((D, C), np.float16)
        xTe[:, :n] = xtT[:, idxs[e]]
        cfull = np.zeros(C, np.float32)
        cfull[:n] = coefs[e]
        coef2d = np.ascontiguousarray(cfull.reshape(C // P, P).T)
        # [ht, half, dp, do*z]: per-(ht, half) slices are contiguous
        # 2 KiB lines per partition
        w13 = np.empty((HT, 2, P, DSUB * P), np.float16)
        w13[:, 0] = w1[e].reshape(DSUB, P, HT, P).transpose(2, 1, 0, 3) \
                         .reshape(HT, P, DSUB * P)
        w13[:, 1] = w3[e].reshape(DSUB, P, HT, P).transpose(2, 1, 0, 3) \
                         .reshape(HT, P, DSUB * P)
        in_maps.append({
            "xT": xTe,
            "w13": w13,
            "w2": np.ascontiguousarray(w2[e].astype(np.float16)),
            "coef": coef2d,
        })

    res = run_bass_kernel_spmd(nc, in_maps, core_ids=list(range(NCORES)))

    out = np.zeros((T, D), np.float32)
    for e in range(E):
        n = len(idxs[e])
        out[idxs[e]] += res.results[e]["y"][:n].astype(np.float32)
    return out.reshape(b, s, d)
